# revision 8
# baseline (speedup 1.0000x reference)
"""MultiHeadDifferentialAttention TRN2 kernel (8 NeuronCores).

Sharding: core k handles batch b = k//2 and head-group g = k%2 (8 of 16 heads).
Everything on-device is computed in "transposed" orientation:
  - scores S^T [m, n] (keys on partitions) -> exp -> E^T (f32r)
  - U^T = v^T @ E^T [d, n] per map; softmax sums via ones-lhsT row-matmuls
  - o_preT = U1^T * (1/s1)[n] - U2^T * (lam/s2)[n]   (broadcast tiles along n)
  - GroupNorm (mean/var over the head) applied as per-head scalars
  - out^T[e, n] = woA^T @ o_gnT accumulated over heads (+ beta@Wo bias)
  - attn^T[m, n] = E1^T*(1/s1)[n] - E2^T*(lam/s2)[n], written per head;
    host transposes at unshard time.

Host folds: score scale (1/8) into Wq, gamma into Wo rows, beta@Wo into a bias.
"""

import numpy as np

import concourse.bass as bass
import concourse.mybir as mybir
from concourse import bacc
from concourse.tile import TileContext
from concourse.bass_utils import run_bass_kernel_spmd

F32 = mybir.dt.float32
F32R = mybir.dt.float32r
AF = mybir.ActivationFunctionType
OP = mybir.AluOpType

E = 1024      # emb dim
H = 8         # heads per core
HD = 128      # 2*d per head
N = 1024      # seq len
NT = 8        # 128-row tiles per 1024
LAMBDA_INIT = 0.8
EPS = 1e-5
N_ELEM = float(N * HD)   # groupnorm element count per (b, h)

_CACHED_NC = None


def build_nc():
    nc = bacc.Bacc(None, target_bir_lowering=False)

    xT = nc.dram_tensor("xT", [E, N], F32, kind="ExternalInput")
    wq = nc.dram_tensor("wq", [E, E], F32, kind="ExternalInput")
    wk = nc.dram_tensor("wk", [E, E], F32, kind="ExternalInput")
    wv = nc.dram_tensor("wv", [E, E], F32, kind="ExternalInput")
    woA = nc.dram_tensor("woA", [E, E], F32, kind="ExternalInput")
    bias_o = nc.dram_tensor("bias_o", [128, 8], F32, kind="ExternalInput")
    lam = nc.dram_tensor("lam", [1, 8], F32, kind="ExternalInput")

    attn_t = nc.dram_tensor("attn_t", [H, N, N], F32, kind="ExternalOutput")
    out_t = nc.dram_tensor("out_t", [E, N], F32, kind="ExternalOutput")

    v_dram = nc.dram_tensor("v_dram", [N, E], F32)          # v (f32r bits), [m, c]
    uT_dram = nc.dram_tensor("uT_dram", [H, HD, N], F32)    # o_preT per head
    srow_dram = nc.dram_tensor("srow_dram", [H, 2, N], F32)  # raw sums rows
    scol_dram = nc.dram_tensor("scol_dram", [H, 2, N], F32)  # 1/s1, lam/s2 (n-order)
    stats_dram = nc.dram_tensor("stats_dram", [1, 16], F32)  # per-head sum/sumsq totals

    with TileContext(nc) as tc:
        with tc.tile_pool(name="const", bufs=1) as cpool, \
             tc.tile_pool(name="persist", bufs=1) as pers, \
             tc.tile_pool(name="ps_s", bufs=1, space="PSUM") as ps_s, \
             tc.tile_pool(name="ps_u", bufs=1, space="PSUM") as ps_u, \
             tc.tile_pool(name="ps_r", bufs=1, space="PSUM") as ps_r:

            # ---- constants ----
            ones_f = cpool.tile([128, 16], F32)
            nc.vector.memset(ones_f[:], 1.0)
            ones16 = cpool.tile([128, 16], F32R)
            nc.vector.tensor_copy(ones16[:], ones_f[:])
            lam_bc = cpool.tile([128, 8], F32)
            nc.sync.dma_start(out=lam_bc, in_=lam[:].broadcast_to([128, 8]))
            bias_sb = cpool.tile([128, 8], F32)
            nc.sync.dma_start(out=bias_sb, in_=bias_o[:])
            stats_all = cpool.tile([128, 16], F32)
            nc.vector.memset(stats_all[:], 0.0)

            # ---- persistent big tensors ----
            qT_sb = pers.tile([128, 8, N], F32R)   # [e128 -> c part][head][n]
            kT_sb = pers.tile([128, 8, N], F32R)

            # ================= P1: qT, kT, v =================
            with tc.tile_pool(name="p1", bufs=2) as p1pool:
                xT_sb = p1pool.tile([128, 8, N], F32R, name="xT_sb", tag="xT", bufs=1)
                nc.gpsimd.dma_start(
                    out=xT_sb, in_=xT.rearrange("(et p) n -> p et n", p=128))

                rot = [(ps_s, "s", 2048), (ps_u, "u", 1024), (ps_r, "r", 1024)]
                ridx = 0

                def p1_psum(name):
                    nonlocal ridx
                    pool, tag, width = rot[ridx % 3]
                    ridx += 1
                    return pool.tile([128, width], F32, tag=tag, name=name)

                for w_dram, dst in ((wq, qT_sb), (wk, kT_sb)):
                    wname = "q" if dst is qT_sb else "k"
                    for ct in range(8):
                        wt = p1pool.tile([128, 8, 128], F32R, tag="wt",
                                         name=f"wt{wname}{ct}")
                        nc.gpsimd.dma_start(
                            out=wt,
                            in_=w_dram.rearrange("(et p) c -> p et c", p=128)[
                                :, :, ct * 128:(ct + 1) * 128])
                        for nch in range(2):
                            pq = p1_psum(f"pq{wname}{ct}{nch}")
                            for et in range(8):
                                nc.tensor.matmul(
                                    pq[:, 0:512], wt[:, et, :],
                                    xT_sb[:, et, nch * 512:(nch + 1) * 512],
                                    start=(et == 0), stop=(et == 7))
                            nc.scalar.activation(
                                dst[:, ct, nch * 512:(nch + 1) * 512], pq[:, 0:512],
                                AF.Copy)
                # v = x @ Wv : lhsT = xT e-tiles, rhs = wv
                for cch in range(2):
                    wt = p1pool.tile([128, 8, 512], F32R, tag="wtv", name=f"wtv{cch}")
                    nc.gpsimd.dma_start(
                        out=wt,
                        in_=wv.rearrange("(et p) c -> p et c", p=128)[
                            :, :, cch * 512:(cch + 1) * 512])
                    for mt in range(8):
                        pv = p1_psum(f"pv{cch}{mt}")
                        for et in range(8):
                            nc.tensor.matmul(
                                pv[:, 0:512],
                                xT_sb[:, et, mt * 128:(mt + 1) * 128],
                                wt[:, et, :],
                                start=(et == 0), stop=(et == 7))
                        vs = p1pool.tile([128, 512], F32R, tag="vs", name=f"vs{cch}{mt}")
                        nc.vector.tensor_copy(vs[:], pv[:, 0:512])
                        nc.sync.dma_start(
                            out=v_dram[mt * 128:(mt + 1) * 128,
                                       cch * 512:(cch + 1) * 512],
                            in_=vs[:].bitcast(F32))

            # ================= per-head attention =================
            with tc.tile_pool(name="epool", bufs=8) as epool, \
                 tc.tile_pool(name="hpool", bufs=2) as hpool, \
                 tc.tile_pool(name="spool", bufs=2) as spool:
                for h in range(H):
                    # ---- scores + exp (per m-tile) ----
                    e_tiles = []
                    for mt in range(NT):
                        ps = ps_s.tile([128, 2048], F32, tag="s", name=f"s{h}{mt}")
                        for i in range(2):
                            ksl = kT_sb[64 * i:64 * i + 64, h, mt * 128:(mt + 1) * 128]
                            for nch in range(2):
                                nc.tensor.matmul(
                                    ps[:, i * 1024 + nch * 512: i * 1024 + nch * 512 + 512],
                                    ksl,
                                    qT_sb[64 * i:64 * i + 64, h,
                                          nch * 512:(nch + 1) * 512],
                                    start=True, stop=True, tile_position=(64 * i, 0))
                        et = epool.tile([128, 2048], F32R, tag="e", name=f"e{h}{mt}")
                        nc.scalar.activation(et[:], ps[:], AF.Exp)
                        e_tiles.append(et)

                    # ---- load v-slice for this head ----
                    vo = hpool.tile([128, 8, 128], F32R, tag="vo", name=f"vo{h}", bufs=1)
                    nc.sync.dma_start(
                        out=vo,
                        in_=v_dram.rearrange("(mt p) c -> p mt c", p=128)[
                            :, :, h * 128:(h + 1) * 128].bitcast(F32R))

                    # ---- AV + sums, map-sequential ----
                    u1_raw = hpool.tile([128, N], F32, tag="u1", name=f"u1{h}", bufs=1)
                    u2_ps = None
                    for i in range(2):
                        pu = ps_u.tile([128, 1024], F32, tag="u", name=f"u{h}{i}")
                        pr = ps_r.tile([128, 1024], F32, tag="r", name=f"r{h}{i}")
                        for mt in range(NT):
                            for nch in range(2):
                                esl = e_tiles[mt][:, i * 1024 + nch * 512:
                                                  i * 1024 + nch * 512 + 512]
                                nc.tensor.matmul(
                                    pu[:, nch * 512:(nch + 1) * 512],
                                    vo[:, mt, :], esl,
                                    start=(mt == 0), stop=(mt == NT - 1))
                                nc.tensor.matmul(
                                    pr[0:16, nch * 512:(nch + 1) * 512],
                                    ones16[:], esl,
                                    start=(mt == 0), stop=(mt == NT - 1))
                        # raw sums row -> dram hop
                        srow = spool.tile([16, N], F32, tag="srow", name=f"sr{h}{i}")
                        nc.vector.tensor_copy(srow[:], pr[0:16, :])
                        nc.sync.dma_start(out=srow_dram[h, i, :][None, :],
                                          in_=srow[0:1, :])
                        if i == 0:
                            nc.vector.tensor_copy(u1_raw[:], pu[:])
                        else:
                            u2_ps = pu

                    # ---- scales: reload, recip, lam, store, broadcast ----
                    scol8 = spool.tile([128, 2, 8], F32, tag="scol8", name=f"s8{h}")
                    for i in range(2):
                        nc.sync.dma_start(
                            out=scol8[:, i, :],
                            in_=srow_dram[h, i, :].rearrange("(j p) -> p j", p=128))
                    rcol = spool.tile([128, 2, 8], F32, tag="rcol", name=f"rc{h}")
                    nc.vector.reciprocal(rcol[:, 0, :], scol8[:, 0, :])
                    nc.vector.reciprocal(rcol[:, 1, :], scol8[:, 1, :])
                    nc.vector.tensor_scalar(
                        out=rcol[:, 1, :], in0=rcol[:, 1, :],
                        scalar1=lam_bc[:, h:h + 1], scalar2=None, op0=OP.mult)
                    for i in range(2):
                        nc.sync.dma_start(
                            out=scol_dram[h, i, :].rearrange("(j p) -> p j", p=128),
                            in_=rcol[:, i, :])
                    s1i_bc = hpool.tile([128, N], F32, tag="s1b", name=f"s1b{h}", bufs=1)
                    r2_bc = hpool.tile([128, N], F32, tag="r2b", name=f"r2b{h}", bufs=1)
                    nc.sync.dma_start(
                        out=s1i_bc, in_=scol_dram[h, 0, :][None, :].broadcast_to([128, N]))
                    nc.sync.dma_start(
                        out=r2_bc, in_=scol_dram[h, 1, :][None, :].broadcast_to([128, N]))

                    # ---- U combine + stats ----
                    t2u = hpool.tile([128, N], F32, tag="t2u", name=f"t2u{h}", bufs=1)
                    nc.vector.tensor_tensor(t2u[:], u2_ps[:], r2_bc[:], op=OP.mult)
                    u1s = hpool.tile([128, N], F32, tag="u1s", name=f"u1s{h}", bufs=1)
                    nc.vector.tensor_tensor(u1s[:], u1_raw[:], s1i_bc[:], op=OP.mult)
                    o_preT = hpool.tile([128, N], F32, tag="opre", name=f"op{h}")
                    nc.vector.scalar_tensor_tensor(
                        out=o_preT[:], in0=u1s[:], scalar=1.0, in1=t2u[:],
                        op0=OP.mult, op1=OP.subtract,
                        accum_out=stats_all[:, 2 * h:2 * h + 1])
                    sq_scr = hpool.tile([128, N], F32, tag="t1a", name=f"sq{h}", bufs=1)
                    nc.scalar.activation(
                        sq_scr[:], o_preT[:], AF.Square,
                        accum_out=stats_all[:, 2 * h + 1:2 * h + 2])
                    nc.sync.dma_start(out=uT_dram[h], in_=o_preT[:])

                    # ---- attn combine (per m-tile) ----
                    for mt in range(NT):
                        t2a = hpool.tile([128, N], F32, tag="t2a", name=f"t2a{h}{mt}")
                        nc.gpsimd.tensor_tensor(
                            t2a[:], e_tiles[mt][:, 1024:2048].bitcast(F32),
                            r2_bc[:], op=OP.mult)
                        t1a = hpool.tile([128, N], F32, tag="t1a", name=f"t1a{h}{mt}", bufs=1)
                        nc.vector.tensor_tensor(
                            t1a[:], e_tiles[mt][:, 0:1024].bitcast(F32),
                            s1i_bc[:], op=OP.mult)
                        at = hpool.tile([128, N], F32, tag="at", name=f"at{h}{mt}")
                        nc.vector.scalar_tensor_tensor(
                            out=at[:], in0=t1a[:], scalar=1.0, in1=t2a[:],
                            op0=OP.mult, op1=OP.subtract)
                        nc.sync.dma_start(
                            out=attn_t[h, mt * 128:(mt + 1) * 128, :], in_=at[:])

            # ================= GN stats + outproj =================
            with tc.tile_pool(name="opool", bufs=1) as opool, \
                 tc.tile_pool(name="ostage", bufs=2) as ostage:
                ones2_f = ostage.tile([128, 2], F32, tag="o2f")
                nc.vector.memset(ones2_f[:], 1.0)
                pstat = ps_u.tile([128, 1024], F32, tag="u", name="pstat")
                nc.tensor.matmul(pstat[0:16, 0:2], stats_all[:], ones2_f[:],
                                 start=True, stop=True)
                tot = ostage.tile([16, 2], F32, tag="tot")
                nc.vector.tensor_copy(tot[:], pstat[0:16, 0:2])
                nc.sync.dma_start(out=stats_dram[0, :][:, None], in_=tot[0:16, 0:1])
                statsb = ostage.tile([128, 16], F32, tag="stb")
                nc.sync.dma_start(out=statsb,
                                  in_=stats_dram[:].broadcast_to([128, 16]))
                mean = ostage.tile([128, 8], F32, tag="mean")
                nc.vector.tensor_scalar(
                    out=mean[:], in0=statsb[:, 0:16:2], scalar1=1.0 / N_ELEM,
                    scalar2=None, op0=OP.mult)
                ex2 = ostage.tile([128, 8], F32, tag="ex2")
                nc.vector.tensor_scalar(
                    out=ex2[:], in0=statsb[:, 1:16:2], scalar1=1.0 / N_ELEM,
                    scalar2=None, op0=OP.mult)
                var = ostage.tile([128, 8], F32, tag="var")
                nc.vector.tensor_tensor(var[:], mean[:], mean[:], op=OP.mult)
                nc.vector.scalar_tensor_tensor(
                    out=var[:], in0=var[:], scalar=-1.0, in1=ex2[:],
                    op0=OP.mult, op1=OP.add)
                nc.vector.tensor_scalar(
                    out=var[:], in0=var[:], scalar1=1.0, scalar2=EPS,
                    op0=OP.mult, op1=OP.add)
                sig = ostage.tile([128, 8], F32, tag="sig")
                nc.scalar.activation(sig[:], var[:], AF.Sqrt)
                # one Newton step: sig = 0.5*(sig + var/sig)
                rs = ostage.tile([128, 8], F32, tag="rs")
                nc.vector.reciprocal(rs[:], sig[:])
                t_nr = ostage.tile([128, 8], F32, tag="tnr")
                nc.vector.tensor_tensor(t_nr[:], var[:], rs[:], op=OP.mult)
                nc.vector.scalar_tensor_tensor(
                    out=sig[:], in0=sig[:], scalar=1.0, in1=t_nr[:],
                    op0=OP.mult, op1=OP.add)
                nc.vector.tensor_scalar(
                    out=sig[:], in0=sig[:], scalar1=0.5, scalar2=None, op0=OP.mult)
                sinv = ostage.tile([128, 8], F32, tag="sinv")
                nc.vector.reciprocal(sinv[:], sig[:])

                # o_gnT per head (f32r) + woA tiles
                ogn = opool.tile([128, 8, N], F32R, name="ogn")
                for h in range(H):
                    raw = ostage.tile([128, N], F32, tag="raw", name=f"raw{h}")
                    nc.sync.dma_start(out=raw, in_=uT_dram[h])
                    nc.vector.tensor_scalar(
                        out=ogn[:, h, :], in0=raw[:],
                        scalar1=mean[:, h:h + 1], scalar2=sinv[:, h:h + 1],
                        op0=OP.subtract, op1=OP.mult)
                woA_sb = opool.tile([128, 8, 8, 128], F32R, name="woA_sb")
                nc.gpsimd.dma_start(
                    out=woA_sb,
                    in_=woA.rearrange("(h p) (et eo) -> p h et eo", p=128, eo=128))
                for et in range(8):
                    for nch in range(2):
                        po = ps_r.tile([128, 1024], F32, tag="r", name=f"po{et}{nch}")
                        for h in range(8):
                            nc.tensor.matmul(
                                po[:, 0:512],
                                woA_sb[:, h, et, :],
                                ogn[:, h, nch * 512:(nch + 1) * 512],
                                start=(h == 0), stop=(h == 7))
                        ot = ostage.tile([128, 512], F32, tag="ot", name=f"ot{et}{nch}")
                        nc.vector.tensor_scalar(
                            out=ot[:], in0=po[:, 0:512],
                            scalar1=bias_sb[:, et:et + 1], scalar2=None, op0=OP.add)
                        nc.sync.dma_start(
                            out=out_t[et * 128:(et + 1) * 128,
                                      nch * 512:(nch + 1) * 512],
                            in_=ot[:])

    nc.finalize()
    return nc


def _get_nc():
    global _CACHED_NC
    if _CACHED_NC is None:
        _CACHED_NC = build_nc()
    return _CACHED_NC


def kernel(x, Wq, Wk, Wv, lq1, lk1, lq2, lk2, gamma, beta, Wo):
    x = np.asarray(x, dtype=np.float32)
    Wq = np.asarray(Wq, dtype=np.float32)
    Wk = np.asarray(Wk, dtype=np.float32)
    Wv = np.asarray(Wv, dtype=np.float32)
    Wo = np.asarray(Wo, dtype=np.float32)
    gamma = np.asarray(gamma, dtype=np.float32)
    beta = np.asarray(beta, dtype=np.float32)
    lam_full = (np.exp(np.sum(np.asarray(lq1) * np.asarray(lk1), axis=1))
                - np.exp(np.sum(np.asarray(lq2) * np.asarray(lk2), axis=1))
                + LAMBDA_INIT).astype(np.float32)          # (16,)

    B, Nn, Ee = x.shape
    gamma_t = np.tile(gamma, H)                             # (1024,)
    beta_t = np.tile(beta, H)

    in_maps = []
    for k in range(8):
        b, g = k // 2, k % 2
        cols = slice(g * 1024, (g + 1) * 1024)
        wo_slice = Wo[cols, :]                              # rows of Wo
        in_maps.append(dict(
            xT=np.ascontiguousarray(x[b].T),
            wq=np.ascontiguousarray(Wq[:, cols] * 0.125),
            wk=np.ascontiguousarray(Wk[:, cols]),
            wv=np.ascontiguousarray(Wv[:, cols]),
            woA=np.ascontiguousarray(wo_slice * gamma_t[:, None]),
            bias_o=np.ascontiguousarray(
                (beta_t @ wo_slice).reshape(8, 128).T),
            lam=np.ascontiguousarray(lam_full[g * 8:(g + 1) * 8][None, :]),
        ))

    nc = _get_nc()
    results = run_bass_kernel_spmd(nc, in_maps, core_ids=list(range(8))).results

    out = np.empty((B, Nn, Ee), dtype=np.float32)
    attn = np.empty((B, 16, Nn, Nn), dtype=np.float32)
    for k in range(8):
        b, g = k // 2, k % 2
        if g == 0:
            out[b] = results[k]["out_t"].T
        else:
            out[b] += results[k]["out_t"].T
        at = results[k]["attn_t"]                           # [8, m, n]
        attn[b, g * 8:(g + 1) * 8] = at.transpose(0, 2, 1)
    return out, attn


# revision 12
# speedup vs baseline: 1.2062x; 1.2062x over previous
"""MultiHeadDifferentialAttention TRN2 kernel (8 NeuronCores).

Sharding: core k handles batch b = k//2 and head-group g = k%2 (8 of 16 heads).
Everything on-device is computed in "transposed" orientation:
  - scores S^T [m, n] (keys on partitions) -> exp -> E^T (f32r)
  - U^T = v^T @ E^T [d, n] per map; softmax sums via ones-lhsT row-matmuls
  - o_preT = U1^T * (1/s1)[n] - U2^T * (lam/s2)[n]   (broadcast tiles along n)
  - GroupNorm (mean/var over the head) applied as per-head scalars
  - out^T[e, n] = woA^T @ o_gnT accumulated over heads (+ beta@Wo bias)
  - attn^T[m, n] = E1^T*(1/s1)[n] - E2^T*(lam/s2)[n], written per head;
    host transposes at unshard time.

Host folds: score scale (1/8) into Wq, gamma into Wo rows, beta@Wo into a bias.
"""

import numpy as np

import jax
import jax.numpy as jnp
from jax.experimental.shard_map import shard_map
from jax.sharding import Mesh, PartitionSpec

import concourse.bass as bass
import concourse.mybir as mybir
from concourse import bacc
from concourse import bass2jax
from concourse.tile import TileContext

F32 = mybir.dt.float32
F32R = mybir.dt.float32r
AF = mybir.ActivationFunctionType
OP = mybir.AluOpType

E = 1024      # emb dim
H = 8         # heads per core
HD = 128      # 2*d per head
N = 1024      # seq len
NT = 8        # 128-row tiles per 1024
LAMBDA_INIT = 0.8
EPS = 1e-5
N_ELEM = float(N * HD)   # groupnorm element count per (b, h)

_CACHED_NC = None


def build_nc():
    nc = bacc.Bacc(None, target_bir_lowering=False)

    xT = nc.dram_tensor("xT", [E, N], F32, kind="ExternalInput")
    wq = nc.dram_tensor("wq", [E, E], F32, kind="ExternalInput")
    wk = nc.dram_tensor("wk", [E, E], F32, kind="ExternalInput")
    wv = nc.dram_tensor("wv", [E, E], F32, kind="ExternalInput")
    woA = nc.dram_tensor("woA", [E, E], F32, kind="ExternalInput")
    bias_o = nc.dram_tensor("bias_o", [128, 8], F32, kind="ExternalInput")
    lam = nc.dram_tensor("lam", [1, 8], F32, kind="ExternalInput")

    attn_t = nc.dram_tensor("attn_t", [H, N, N], F32, kind="ExternalOutput")
    out_t = nc.dram_tensor("out_t", [E, N], F32, kind="ExternalOutput")

    v_dram = nc.dram_tensor("v_dram", [N, E], F32)          # v (f32r bits), [m, c]
    uT_dram = nc.dram_tensor("uT_dram", [H, HD, N], F32)    # o_preT per head
    srow_dram = nc.dram_tensor("srow_dram", [H, 2, N], F32)  # raw sums rows
    scol_dram = nc.dram_tensor("scol_dram", [H, 2, N], F32)  # 1/s1, lam/s2 (n-order)
    stats_dram = nc.dram_tensor("stats_dram", [1, 16], F32)  # per-head sum/sumsq totals

    with TileContext(nc) as tc:
        with tc.tile_pool(name="const", bufs=1) as cpool, \
             tc.tile_pool(name="persist", bufs=1) as pers, \
             tc.tile_pool(name="ps_s", bufs=1, space="PSUM") as ps_s, \
             tc.tile_pool(name="ps_u", bufs=1, space="PSUM") as ps_u, \
             tc.tile_pool(name="ps_r", bufs=1, space="PSUM") as ps_r:

            # ---- constants ----
            ones_f = cpool.tile([128, 16], F32)
            nc.vector.memset(ones_f[:], 1.0)
            ones16 = cpool.tile([128, 16], F32R)
            nc.vector.tensor_copy(ones16[:], ones_f[:])
            lam_bc = cpool.tile([128, 8], F32)
            nc.sync.dma_start(out=lam_bc, in_=lam[:].broadcast_to([128, 8]))
            bias_sb = cpool.tile([128, 8], F32)
            nc.sync.dma_start(out=bias_sb, in_=bias_o[:])
            stats_all = cpool.tile([128, 16], F32)
            nc.vector.memset(stats_all[:], 0.0)

            # ---- persistent big tensors ----
            qT_sb = pers.tile([128, 8, N], F32R)   # [e128 -> c part][head][n]
            kT_sb = pers.tile([128, 8, N], F32R)

            # ================= P1: qT, kT, v =================
            with tc.tile_pool(name="p1", bufs=2) as p1pool:
                xT_sb = p1pool.tile([128, 8, N], F32R, name="xT_sb", tag="xT", bufs=1)
                nc.gpsimd.dma_start(
                    out=xT_sb, in_=xT.rearrange("(et p) n -> p et n", p=128))

                rot = [(ps_s, "s", 2048), (ps_u, "u", 1024), (ps_r, "r", 1024)]
                ridx = 0

                def p1_psum(name):
                    nonlocal ridx
                    pool, tag, width = rot[ridx % 3]
                    ridx += 1
                    return pool.tile([128, width], F32, tag=tag, name=name)

                for w_dram, dst in ((wq, qT_sb), (wk, kT_sb)):
                    wname = "q" if dst is qT_sb else "k"
                    for ct in range(8):
                        wt = p1pool.tile([128, 8, 128], F32R, tag="wt",
                                         name=f"wt{wname}{ct}")
                        nc.gpsimd.dma_start(
                            out=wt,
                            in_=w_dram.rearrange("(et p) c -> p et c", p=128)[
                                :, :, ct * 128:(ct + 1) * 128])
                        for nch in range(2):
                            pq = p1_psum(f"pq{wname}{ct}{nch}")
                            for et in range(8):
                                nc.tensor.matmul(
                                    pq[:, 0:512], wt[:, et, :],
                                    xT_sb[:, et, nch * 512:(nch + 1) * 512],
                                    start=(et == 0), stop=(et == 7))
                            nc.scalar.activation(
                                dst[:, ct, nch * 512:(nch + 1) * 512], pq[:, 0:512],
                                AF.Copy)
                # v = x @ Wv : lhsT = xT e-tiles, rhs = wv
                for cch in range(2):
                    wt = p1pool.tile([128, 8, 512], F32R, tag="wtv", name=f"wtv{cch}")
                    nc.gpsimd.dma_start(
                        out=wt,
                        in_=wv.rearrange("(et p) c -> p et c", p=128)[
                            :, :, cch * 512:(cch + 1) * 512])
                    for mt in range(8):
                        pv = p1_psum(f"pv{cch}{mt}")
                        for et in range(8):
                            nc.tensor.matmul(
                                pv[:, 0:512],
                                xT_sb[:, et, mt * 128:(mt + 1) * 128],
                                wt[:, et, :],
                                start=(et == 0), stop=(et == 7))
                        vs = p1pool.tile([128, 512], F32R, tag="vs", name=f"vs{cch}{mt}")
                        nc.vector.tensor_copy(vs[:], pv[:, 0:512])
                        nc.sync.dma_start(
                            out=v_dram[mt * 128:(mt + 1) * 128,
                                       cch * 512:(cch + 1) * 512],
                            in_=vs[:].bitcast(F32))

            # ================= per-head attention =================
            with tc.tile_pool(name="epool", bufs=8) as epool, \
                 tc.tile_pool(name="hpool", bufs=2) as hpool, \
                 tc.tile_pool(name="spool", bufs=2) as spool:
                for h in range(H):
                    # ---- scores + exp (per m-tile) ----
                    e_tiles = []
                    for mt in range(NT):
                        ps = ps_s.tile([128, 2048], F32, tag="s", name=f"s{h}{mt}")
                        for i in range(2):
                            ksl = kT_sb[64 * i:64 * i + 64, h, mt * 128:(mt + 1) * 128]
                            for nch in range(2):
                                nc.tensor.matmul(
                                    ps[:, i * 1024 + nch * 512: i * 1024 + nch * 512 + 512],
                                    ksl,
                                    qT_sb[64 * i:64 * i + 64, h,
                                          nch * 512:(nch + 1) * 512],
                                    start=True, stop=True, tile_position=(64 * i, 0))
                        et = epool.tile([128, 2048], F32R, tag="e", name=f"e{h}{mt}")
                        nc.scalar.activation(et[:], ps[:], AF.Exp)
                        e_tiles.append(et)

                    # ---- load v-slice for this head ----
                    vo = hpool.tile([128, 8, 128], F32R, tag="vo", name=f"vo{h}", bufs=1)
                    nc.sync.dma_start(
                        out=vo,
                        in_=v_dram.rearrange("(mt p) c -> p mt c", p=128)[
                            :, :, h * 128:(h + 1) * 128].bitcast(F32R))

                    # ---- AV + sums, map-sequential ----
                    u1_raw = hpool.tile([128, N], F32, tag="u1", name=f"u1{h}", bufs=1)
                    u2_ps = None
                    for i in range(2):
                        pu = ps_u.tile([128, 1024], F32, tag="u", name=f"u{h}{i}")
                        pr = ps_r.tile([128, 1024], F32, tag="r", name=f"r{h}{i}")
                        for mt in range(NT):
                            for nch in range(2):
                                esl = e_tiles[mt][:, i * 1024 + nch * 512:
                                                  i * 1024 + nch * 512 + 512]
                                nc.tensor.matmul(
                                    pu[:, nch * 512:(nch + 1) * 512],
                                    vo[:, mt, :], esl,
                                    start=(mt == 0), stop=(mt == NT - 1))
                                nc.tensor.matmul(
                                    pr[0:16, nch * 512:(nch + 1) * 512],
                                    ones16[:], esl,
                                    start=(mt == 0), stop=(mt == NT - 1))
                        # raw sums row -> dram hop
                        srow = spool.tile([16, N], F32, tag="srow", name=f"sr{h}{i}")
                        nc.vector.tensor_copy(srow[:], pr[0:16, :])
                        nc.sync.dma_start(out=srow_dram[h, i, :][None, :],
                                          in_=srow[0:1, :])
                        if i == 0:
                            nc.vector.tensor_copy(u1_raw[:], pu[:])
                        else:
                            u2_ps = pu

                    # ---- scales: reload, recip, lam, store, broadcast ----
                    scol8 = spool.tile([128, 2, 8], F32, tag="scol8", name=f"s8{h}")
                    for i in range(2):
                        nc.sync.dma_start(
                            out=scol8[:, i, :],
                            in_=srow_dram[h, i, :].rearrange("(j p) -> p j", p=128))
                    rcol = spool.tile([128, 2, 8], F32, tag="rcol", name=f"rc{h}")
                    nc.vector.reciprocal(rcol[:, 0, :], scol8[:, 0, :])
                    nc.vector.reciprocal(rcol[:, 1, :], scol8[:, 1, :])
                    nc.vector.tensor_scalar(
                        out=rcol[:, 1, :], in0=rcol[:, 1, :],
                        scalar1=lam_bc[:, h:h + 1], scalar2=None, op0=OP.mult)
                    for i in range(2):
                        nc.sync.dma_start(
                            out=scol_dram[h, i, :].rearrange("(j p) -> p j", p=128),
                            in_=rcol[:, i, :])
                    s1i_bc = hpool.tile([128, N], F32, tag="s1b", name=f"s1b{h}", bufs=1)
                    r2_bc = hpool.tile([128, N], F32, tag="r2b", name=f"r2b{h}", bufs=1)
                    nc.sync.dma_start(
                        out=s1i_bc, in_=scol_dram[h, 0, :][None, :].broadcast_to([128, N]))
                    nc.sync.dma_start(
                        out=r2_bc, in_=scol_dram[h, 1, :][None, :].broadcast_to([128, N]))

                    # ---- U combine + stats ----
                    t2u = hpool.tile([128, N], F32, tag="t2u", name=f"t2u{h}", bufs=1)
                    nc.vector.tensor_tensor(t2u[:], u2_ps[:], r2_bc[:], op=OP.mult)
                    u1s = hpool.tile([128, N], F32, tag="u1s", name=f"u1s{h}", bufs=1)
                    nc.vector.tensor_tensor(u1s[:], u1_raw[:], s1i_bc[:], op=OP.mult)
                    o_preT = hpool.tile([128, N], F32, tag="opre", name=f"op{h}")
                    nc.vector.scalar_tensor_tensor(
                        out=o_preT[:], in0=u1s[:], scalar=1.0, in1=t2u[:],
                        op0=OP.mult, op1=OP.subtract,
                        accum_out=stats_all[:, 2 * h:2 * h + 1])
                    sq_scr = hpool.tile([128, N], F32, tag="t1a", name=f"sq{h}", bufs=1)
                    nc.scalar.activation(
                        sq_scr[:], o_preT[:], AF.Square,
                        accum_out=stats_all[:, 2 * h + 1:2 * h + 2])
                    nc.sync.dma_start(out=uT_dram[h], in_=o_preT[:])

                    # ---- attn combine (per m-tile) ----
                    for mt in range(NT):
                        t2a = hpool.tile([128, N], F32, tag="t2a", name=f"t2a{h}{mt}")
                        nc.gpsimd.tensor_tensor(
                            t2a[:], e_tiles[mt][:, 1024:2048].bitcast(F32),
                            r2_bc[:], op=OP.mult)
                        t1a = hpool.tile([128, N], F32, tag="t1a", name=f"t1a{h}{mt}", bufs=1)
                        nc.vector.tensor_tensor(
                            t1a[:], e_tiles[mt][:, 0:1024].bitcast(F32),
                            s1i_bc[:], op=OP.mult)
                        at = hpool.tile([128, N], F32, tag="at", name=f"at{h}{mt}")
                        nc.vector.scalar_tensor_tensor(
                            out=at[:], in0=t1a[:], scalar=1.0, in1=t2a[:],
                            op0=OP.mult, op1=OP.subtract)
                        nc.sync.dma_start(
                            out=attn_t[h, mt * 128:(mt + 1) * 128, :], in_=at[:])

            # ================= GN stats + outproj =================
            with tc.tile_pool(name="opool", bufs=1) as opool, \
                 tc.tile_pool(name="ostage", bufs=2) as ostage:
                ones2_f = ostage.tile([128, 2], F32, tag="o2f")
                nc.vector.memset(ones2_f[:], 1.0)
                pstat = ps_u.tile([128, 1024], F32, tag="u", name="pstat")
                nc.tensor.matmul(pstat[0:16, 0:2], stats_all[:], ones2_f[:],
                                 start=True, stop=True)
                tot = ostage.tile([16, 2], F32, tag="tot")
                nc.vector.tensor_copy(tot[:], pstat[0:16, 0:2])
                nc.sync.dma_start(out=stats_dram[0, :][:, None], in_=tot[0:16, 0:1])
                statsb = ostage.tile([128, 16], F32, tag="stb")
                nc.sync.dma_start(out=statsb,
                                  in_=stats_dram[:].broadcast_to([128, 16]))
                mean = ostage.tile([128, 8], F32, tag="mean")
                nc.vector.tensor_scalar(
                    out=mean[:], in0=statsb[:, 0:16:2], scalar1=1.0 / N_ELEM,
                    scalar2=None, op0=OP.mult)
                ex2 = ostage.tile([128, 8], F32, tag="ex2")
                nc.vector.tensor_scalar(
                    out=ex2[:], in0=statsb[:, 1:16:2], scalar1=1.0 / N_ELEM,
                    scalar2=None, op0=OP.mult)
                var = ostage.tile([128, 8], F32, tag="var")
                nc.vector.tensor_tensor(var[:], mean[:], mean[:], op=OP.mult)
                nc.vector.scalar_tensor_tensor(
                    out=var[:], in0=var[:], scalar=-1.0, in1=ex2[:],
                    op0=OP.mult, op1=OP.add)
                nc.vector.tensor_scalar(
                    out=var[:], in0=var[:], scalar1=1.0, scalar2=EPS,
                    op0=OP.mult, op1=OP.add)
                sig = ostage.tile([128, 8], F32, tag="sig")
                nc.scalar.activation(sig[:], var[:], AF.Sqrt)
                # one Newton step: sig = 0.5*(sig + var/sig)
                rs = ostage.tile([128, 8], F32, tag="rs")
                nc.vector.reciprocal(rs[:], sig[:])
                t_nr = ostage.tile([128, 8], F32, tag="tnr")
                nc.vector.tensor_tensor(t_nr[:], var[:], rs[:], op=OP.mult)
                nc.vector.scalar_tensor_tensor(
                    out=sig[:], in0=sig[:], scalar=1.0, in1=t_nr[:],
                    op0=OP.mult, op1=OP.add)
                nc.vector.tensor_scalar(
                    out=sig[:], in0=sig[:], scalar1=0.5, scalar2=None, op0=OP.mult)
                sinv = ostage.tile([128, 8], F32, tag="sinv")
                nc.vector.reciprocal(sinv[:], sig[:])

                # o_gnT per head (f32r) + woA tiles
                ogn = opool.tile([128, 8, N], F32R, name="ogn")
                for h in range(H):
                    raw = ostage.tile([128, N], F32, tag="raw", name=f"raw{h}")
                    nc.sync.dma_start(out=raw, in_=uT_dram[h])
                    nc.vector.tensor_scalar(
                        out=ogn[:, h, :], in0=raw[:],
                        scalar1=mean[:, h:h + 1], scalar2=sinv[:, h:h + 1],
                        op0=OP.subtract, op1=OP.mult)
                woA_sb = opool.tile([128, 8, 8, 128], F32R, name="woA_sb")
                nc.gpsimd.dma_start(
                    out=woA_sb,
                    in_=woA.rearrange("(h p) (et eo) -> p h et eo", p=128, eo=128))
                for et in range(8):
                    for nch in range(2):
                        po = ps_r.tile([128, 1024], F32, tag="r", name=f"po{et}{nch}")
                        for h in range(8):
                            nc.tensor.matmul(
                                po[:, 0:512],
                                woA_sb[:, h, et, :],
                                ogn[:, h, nch * 512:(nch + 1) * 512],
                                start=(h == 0), stop=(h == 7))
                        ot = ostage.tile([128, 512], F32, tag="ot", name=f"ot{et}{nch}")
                        nc.vector.tensor_scalar(
                            out=ot[:], in0=po[:, 0:512],
                            scalar1=bias_sb[:, et:et + 1], scalar2=None, op0=OP.add)
                        nc.sync.dma_start(
                            out=out_t[et * 128:(et + 1) * 128,
                                      nch * 512:(nch + 1) * 512],
                            in_=ot[:])

    nc.finalize()
    return nc


class _Runner:
    """Compile the Bass module through PJRT once; execute many times.

    Mirrors bass2jax.run_bass_via_pjrt's multi-core branch, but (a) caches the
    compiled executable, and (b) materializes the donated output buffers on
    device with jnp.zeros instead of shipping host zeros every call.
    """

    def __init__(self, nc, n_cores=8):
        bass2jax.install_neuronx_cc_hook()
        self.n_cores = n_cores
        partition_name = (nc.partition_id_tensor.name
                          if nc.partition_id_tensor else None)
        in_names, out_names, out_avals = [], [], []
        for alloc in nc.m.functions[0].allocations:
            if not isinstance(alloc, mybir.MemoryLocationSet):
                continue
            name = alloc.memorylocations[0].name
            if alloc.kind == "ExternalInput":
                if name != partition_name:
                    in_names.append(name)
            elif alloc.kind == "ExternalOutput":
                out_names.append(name)
                out_avals.append(jax.core.ShapedArray(
                    tuple(alloc.tensor_shape), mybir.dt.np(alloc.dtype)))
        self.in_names = list(in_names)
        self.out_names = out_names
        self.out_avals = out_avals
        n_params = len(in_names)
        bind_names = in_names + out_names
        if partition_name is not None:
            bind_names.append(partition_name)

        def _body(*args):
            operands = list(args)
            if partition_name is not None:
                operands.append(bass2jax.partition_id_tensor())
            outs = bass2jax._bass_exec_p.bind(
                *operands,
                out_avals=tuple(out_avals),
                in_names=tuple(bind_names),
                out_names=tuple(out_names),
                lowering_input_output_aliases=(),
                sim_require_finite=True,
                sim_require_nnan=True,
                nc=nc,
            )
            return tuple(outs)

        devices = jax.devices()[:n_cores]
        self.mesh = Mesh(np.asarray(devices), ("core",))
        in_specs = (PartitionSpec("core"),) * (n_params + len(out_names))
        out_specs = (PartitionSpec("core"),) * len(out_names)
        self._zeros = [
            jax.device_put(
                np.zeros((n_cores * a.shape[0], *a.shape[1:]), a.dtype),
                jax.sharding.NamedSharding(self.mesh, PartitionSpec("core")))
            for a in out_avals]
        self.fn = jax.jit(shard_map(_body, mesh=self.mesh, in_specs=in_specs,
                                    out_specs=out_specs, check_rep=False))
        self.compiled = None

    def stage(self, in_maps):
        """Concatenate per-core inputs along axis 0 and device_put."""
        n = self.n_cores
        concat = [np.concatenate([np.asarray(in_maps[c][name])
                                  for c in range(n)], axis=0)
                  for name in self.in_names]
        return concat

    def run(self, staged):
        args = list(staged) + self._zeros
        if self.compiled is None:
            self.compiled = self.fn.lower(*args).compile()
        out_arrs = self.compiled(*args)
        jax.block_until_ready(out_arrs)
        return out_arrs

    def split(self, out_arrs):
        n = self.n_cores
        return [
            {name: np.asarray(out_arrs[i]).reshape(n, *self.out_avals[i].shape)[c]
             for i, name in enumerate(self.out_names)}
            for c in range(n)
        ]


_CACHED_RUNNER = None


def _get_runner():
    global _CACHED_RUNNER
    if _CACHED_RUNNER is None:
        _CACHED_RUNNER = _Runner(build_nc(), n_cores=8)
    return _CACHED_RUNNER


def kernel(x, Wq, Wk, Wv, lq1, lk1, lq2, lk2, gamma, beta, Wo):
    x = np.asarray(x, dtype=np.float32)
    Wq = np.asarray(Wq, dtype=np.float32)
    Wk = np.asarray(Wk, dtype=np.float32)
    Wv = np.asarray(Wv, dtype=np.float32)
    Wo = np.asarray(Wo, dtype=np.float32)
    gamma = np.asarray(gamma, dtype=np.float32)
    beta = np.asarray(beta, dtype=np.float32)
    lam_full = (np.exp(np.sum(np.asarray(lq1) * np.asarray(lk1), axis=1))
                - np.exp(np.sum(np.asarray(lq2) * np.asarray(lk2), axis=1))
                + LAMBDA_INIT).astype(np.float32)          # (16,)

    B, Nn, Ee = x.shape
    gamma_t = np.tile(gamma, H)                             # (1024,)
    beta_t = np.tile(beta, H)

    in_maps = []
    for k in range(8):
        b, g = k // 2, k % 2
        cols = slice(g * 1024, (g + 1) * 1024)
        wo_slice = Wo[cols, :]                              # rows of Wo
        in_maps.append(dict(
            xT=np.ascontiguousarray(x[b].T),
            wq=np.ascontiguousarray(Wq[:, cols] * 0.125),
            wk=np.ascontiguousarray(Wk[:, cols]),
            wv=np.ascontiguousarray(Wv[:, cols]),
            woA=np.ascontiguousarray(wo_slice * gamma_t[:, None]),
            bias_o=np.ascontiguousarray(
                (beta_t @ wo_slice).reshape(8, 128).T),
            lam=np.ascontiguousarray(lam_full[g * 8:(g + 1) * 8][None, :]),
        ))

    runner = _get_runner()
    results = runner.split(runner.run(runner.stage(in_maps)))

    out = np.empty((B, Nn, Ee), dtype=np.float32)
    attn = np.empty((B, 16, Nn, Nn), dtype=np.float32)
    for k in range(8):
        b, g = k // 2, k % 2
        if g == 0:
            out[b] = results[k]["out_t"].T
        else:
            out[b] += results[k]["out_t"].T
        at = results[k]["attn_t"]                           # [8, m, n]
        attn[b, g * 8:(g + 1) * 8] = at.transpose(0, 2, 1)
    return out, attn
